# revision 9
# baseline (speedup 1.0000x reference)
"""KoLeo loss (view-expanded) on 8 Trainium2 NeuronCores.

Reference math, per view (T=4 views of X [B=8192, D=1024] fp32):
    xn  = x / ||x||                       (row L2 normalize, fp32)
    m_i = max_{j != i} <xn_i, xn_j>       (masked Gram row max)
    dist_i = ||xn_i - xn_{argmax}|| = sqrt(2 - 2 m_i)   (unit rows; the
             reference's +1e-12 eps terms are < 1e-10 relative -> ignored)
    loss = mean_views( -mean_i log(dist_i) ) = -0.5/(T*B) * sum ln(2 - 2 m_i)

Sharding: data-parallel over query rows. Each of the 8 cores computes a
B/8=1024-row slab of the Gram matrix against all B keys. The input for
core c is np.roll'ed by -c*1024 rows so the (single SPMD) program always
treats rows 0..1023 as its queries and the diagonal mask stays at a
static position.

Per-core device pipeline, per view:
  phase 1 (normalize, row-major): 64 chunks [128,1024] f32 stream from
    DRAM; ScalarE Square+accum_out produces row sums of squares; rsqrt =
    exp(-0.5*ln(n2)) on ScalarE + one fp32 Newton step on VectorE;
    VectorE tensor_scalar scales rows and casts to bf16; chunks stored to
    a DRAM scratch Xn [8192,1024] bf16 (2 scratches, view parity).
  phase 2 (Gram + row max): DMA-transpose loads build K^T panels
    [128(d), 2048(b)] bf16 plus resident Q^T [128,1024] slices; TensorE
    accumulates G slabs into PSUM [128,2048] f32 (8 K-chunks x 4 N=512
    matmuls); VectorE tensor_tensor_reduce adds the -4*I diagonal mask
    (first quarter only) and row-max-reduces; running max across quarters.
  tail: ScalarE Ln(2 - 2*m), VectorE row-sum -> logsum [128,1] per core.
Host: loss = -0.5 * sum(all cores' logsum) / (T*B).
"""

import numpy as np

_B = 8192
_T = 4
_D = 1024
_NCORES = 8

_nc_cache = {}


def build_nc(B=_B, T=_T, D=_D, ncores=_NCORES, enable_asserts=False, debug=False):
    import concourse.tile as tile
    from concourse import bacc, mybir

    P = 128
    NQ = B // ncores          # query rows per core
    MB = NQ // P              # m-blocks per view
    QC = min(2048, B)         # gram columns per quarter (= one PSUM tile)
    NQUART = B // QC
    NB = QC // 512            # matmuls per PSUM tile fill
    KC = D // P               # contraction chunks
    CH = B // P               # row chunks per view
    GRP = min(8, CH)          # chunks per scale batch
    NG = CH // GRP
    MCOLS = T * MB
    assert NQ % P == 0 and B % QC == 0 and QC % 512 == 0 and D % P == 0
    assert CH % GRP == 0 and NQ <= QC  # diagonal always lands in quarter 0

    f32 = mybir.dt.float32
    bf16 = mybir.dt.bfloat16
    AF = mybir.ActivationFunctionType
    ALU = mybir.AluOpType
    AX = mybir.AxisListType

    nc = bacc.Bacc(
        "TRN2",
        target_bir_lowering=False,
        debug=debug,
        enable_asserts=enable_asserts,
    )

    x = nc.dram_tensor("x", [B, T, D], f32, kind="ExternalInput").ap()
    negdiag = nc.dram_tensor("negdiag", [P, P], f32, kind="ExternalInput").ap()
    logsum = nc.dram_tensor("logsum", [P, 1], f32, kind="ExternalOutput").ap()
    maxes = nc.dram_tensor("maxes", [P, MCOLS], f32, kind="ExternalOutput").ap()
    xn = [nc.dram_tensor(f"xn{i}", [B, D], bf16).ap() for i in range(2)]

    with tile.TileContext(nc) as tc:
        with (
            tc.tile_pool(name="consts", bufs=1) as consts,
            tc.tile_pool(name="xin", bufs=10) as xin_pool,
            tc.tile_pool(name="sq", bufs=2) as sq_pool,
            tc.tile_pool(name="xnb", bufs=3) as xnb_pool,
            tc.tile_pool(name="stats", bufs=2) as stats_pool,
            tc.tile_pool(name="small", bufs=4) as small_pool,
            tc.tile_pool(name="qt", bufs=2) as qt_pool,
            tc.tile_pool(name="kt", bufs=2) as kt_pool,
            tc.tile_pool(name="acc", bufs=1) as acc_pool,
            tc.tile_pool(name="ps", bufs=2, space="PSUM") as ps_pool,
        ):
            negd = consts.tile([P, P], f32)
            nc.sync.dma_start(out=negd, in_=negdiag)
            bias2 = consts.tile([P, 1], f32)
            nc.vector.memset(bias2, 2.0)

            mbuf = acc_pool.tile([P, MCOLS], f32)

            for t in range(T):
                xnt = xn[t % 2]

                # ---- phase 1: normalize rows, store bf16 scratch ----
                n2 = stats_pool.tile([P, CH], f32, name=f"n2_{t}", tag="n2")
                sc = stats_pool.tile([P, CH], f32, name=f"sc_{t}", tag="sc")
                for g in range(NG):
                    xins = []
                    for j in range(GRP):
                        bc = g * GRP + j
                        xin_t = xin_pool.tile(
                            [P, D], f32, name=f"xin_{t}_{bc}", tag="xin"
                        )
                        nc.sync.dma_start(
                            out=xin_t, in_=x[bc * P:(bc + 1) * P, t, :]
                        )
                        sqt = sq_pool.tile([P, D], f32, name=f"sq_{t}_{bc}", tag="sq")
                        nc.scalar.activation(
                            out=sqt,
                            in_=xin_t,
                            func=AF.Square,
                            accum_out=n2[:, bc:bc + 1],
                        )
                        xins.append(xin_t)

                    gs = slice(g * GRP, (g + 1) * GRP)
                    # rsqrt seed via exp(-0.5 ln(n2)) (same ACT table set as
                    # Square/Ln), then one fp32 Newton step:
                    #   s = s0 * (1.5 - 0.5 * n2 * s0^2)
                    lnv = small_pool.tile([P, GRP], f32, name=f"lnv_{t}_{g}", tag="lnv")
                    nc.scalar.activation(out=lnv, in_=n2[:, gs], func=AF.Ln)
                    s0 = small_pool.tile([P, GRP], f32, name=f"s0_{t}_{g}", tag="s0")
                    nc.scalar.activation(out=s0, in_=lnv, func=AF.Exp, scale=-0.5)
                    t1 = small_pool.tile([P, GRP], f32, name=f"t1_{t}_{g}", tag="t1")
                    nc.vector.tensor_mul(t1, s0, s0)
                    t2 = small_pool.tile([P, GRP], f32, name=f"t2_{t}_{g}", tag="t2")
                    nc.vector.tensor_mul(t2, t1, n2[:, gs])
                    t3 = small_pool.tile([P, GRP], f32, name=f"t3_{t}_{g}", tag="t3")
                    nc.vector.tensor_scalar(t3, t2, -0.5, 1.5, ALU.mult, ALU.add)
                    nc.vector.tensor_mul(sc[:, gs], s0, t3)

                    for j in range(GRP):
                        bc = g * GRP + j
                        xnb = xnb_pool.tile(
                            [P, D], bf16, name=f"xnb_{t}_{bc}", tag="xnb"
                        )
                        nc.vector.tensor_scalar_mul(xnb, xins[j], sc[:, bc:bc + 1])
                        nc.sync.dma_start(
                            out=xnt[bc * P:(bc + 1) * P, :], in_=xnb
                        )

                # ---- phase 2: Gram slab + masked row max ----
                qts = []
                for k in range(KC):
                    qt_t = qt_pool.tile([P, NQ], bf16, name=f"qt_{t}_{k}", tag=f"qt{k}")
                    nc.sync.dma_start_transpose(
                        out=qt_t, in_=xnt[0:NQ, k * P:(k + 1) * P]
                    )
                    qts.append(qt_t)
                for q in range(NQUART):
                    kts = []
                    for k in range(KC):
                        kt_t = kt_pool.tile(
                            [P, QC], bf16, name=f"kt_{t}_{q}_{k}", tag=f"kt{k}"
                        )
                        nc.sync.dma_start_transpose(
                            out=kt_t, in_=xnt[q * QC:(q + 1) * QC, k * P:(k + 1) * P]
                        )
                        kts.append(kt_t)
                    for mi in range(MB):
                        ps = ps_pool.tile([P, QC], f32, name=f"ps_{t}_{q}_{mi}", tag="ps")
                        for k in range(KC):
                            for nb in range(NB):
                                nc.tensor.matmul(
                                    ps[:, nb * 512:(nb + 1) * 512],
                                    qts[k][:, mi * P:(mi + 1) * P],
                                    kts[k][:, nb * 512:(nb + 1) * 512],
                                    start=(k == 0),
                                    stop=(k == KC - 1),
                                )
                        col = t * MB + mi
                        if q == 0:
                            # mask the self-dot: psum diag window += -4*I,
                            # then row max straight into mbuf
                            nc.vector.tensor_tensor(
                                ps[:, mi * P:(mi + 1) * P],
                                ps[:, mi * P:(mi + 1) * P],
                                negd,
                                op=ALU.add,
                            )
                            nc.vector.reduce_max(
                                mbuf[:, col:col + 1], ps, axis=AX.X
                            )
                        else:
                            qm = small_pool.tile(
                                [P, 1], f32, name=f"qm_{t}_{q}_{mi}", tag="qm"
                            )
                            nc.vector.reduce_max(qm, ps, axis=AX.X)
                            nc.vector.tensor_tensor(
                                mbuf[:, col:col + 1],
                                mbuf[:, col:col + 1],
                                qm,
                                op=ALU.max,
                            )

            # ---- tail: ln(2 - 2m), row-sum ----
            logb = acc_pool.tile([P, MCOLS], f32)
            nc.scalar.activation(out=logb, in_=mbuf, func=AF.Ln, scale=-2.0, bias=bias2)
            lsum = acc_pool.tile([P, 1], f32)
            nc.vector.reduce_sum(lsum, logb, axis=AX.X)
            nc.sync.dma_start(out=logsum, in_=lsum)
            nc.sync.dma_start(out=maxes, in_=mbuf)

    nc.compile()
    return nc


def make_negdiag(B=_B, ncores=_NCORES, maskval=-4.0):
    return (maskval * np.eye(128)).astype(np.float32)


def make_in_maps(x, B=_B, T=_T, D=_D, ncores=_NCORES):
    """x: [B, T, D] fp32 full input -> per-core rolled input maps."""
    x = np.ascontiguousarray(x, dtype=np.float32)
    assert x.shape == (B, T, D)
    nd = make_negdiag(B, ncores)
    NQ = B // ncores
    in_maps = []
    for c in range(ncores):
        xr = np.roll(x, -c * NQ, axis=0) if c else x
        in_maps.append({"x": np.ascontiguousarray(xr), "negdiag": nd})
    return in_maps


def assemble_output(results, B=_B, T=_T):
    total = 0.0
    for r in results:
        total += float(r["logsum"].astype(np.float64).sum())
    loss = -0.5 * total / (T * B)
    return np.asarray(loss, dtype=np.float32)


def kernel(episodes_vectors: np.ndarray) -> np.ndarray:
    from concourse.bass_utils import run_bass_kernel_spmd

    key = (_B, _T, _D, _NCORES)
    if key not in _nc_cache:
        _nc_cache[key] = build_nc()
    nc = _nc_cache[key]

    in_maps = make_in_maps(episodes_vectors)
    last_err = None
    for _attempt in range(3):
        try:
            res = run_bass_kernel_spmd(nc, in_maps, list(range(_NCORES)))
            return assemble_output(res.results)
        except Exception as e:  # transient PJRT/tunnel INTERNAL errors
            last_err = e
    raise last_err


if __name__ == "__main__":
    inputs = {
        "episodes_vectors": np.random.default_rng(0)
        .standard_normal((_B, _T, _D))
        .astype(np.float32)
    }
    print(kernel(**inputs))
